# revision 10
# baseline (speedup 1.0000x reference)
"""GNN attention (GAT-style single-target-node) kernel for 8 Trainium2 cores.

Problem:  x [32, 50000, 64], a [128, 1], node_index scalar, adj_mask [50000]
  tgt_score = x[:, idx] @ a[:64]                             # [B]
  e = leaky_relu(tgt_score[:, None] + x @ a[64:], 0.01)      # [B, N]
  attention = softmax(where(adj>0, e, -9e15), axis=1) * adj  # [B, N]

Sharding: data-parallel over batch (32 = 8 cores x 4 batches/core). Each
core computes complete softmax rows, so no cross-core reductions.

Per-core layout: nodes are tiled [128 partitions, K nodes, 64 feats];
scores accumulate into a [128, 400] grid (12 full tiles of K=32 plus a
[53, 16] tail).  Dot products: elementwise multiply with a replicated
a_src, then a grouped free-axis reduce.  Multiplies are split between
the vector engine and GPSIMD for load balance (DVE also does reduces).
Softmax over the grid uses gpsimd.partition_all_reduce for the
cross-partition max/sum.
"""

import numpy as np
from contextlib import ExitStack

import concourse.bass as bass
import concourse.bacc as bacc
import concourse.tile as tile
from concourse import mybir
from concourse.bass_utils import run_bass_kernel_spmd

B, N, D = 32, 50000, 64
NCORES = 8
BPC = B // NCORES            # batches per core
K = 32                       # nodes per partition per full tile
TFULL = 12                   # full tiles: 12 * 128 * 32 = 49152 nodes
NFULL = TFULL * 128 * K      # 49152
KT = 16                      # tail: nodes per partition
PT = (N - NFULL) // KT       # 53 partitions in tail tile
COLS = TFULL * K + KT        # 400 score columns
NEG = -9.0e15

F32 = mybir.dt.float32
AX = mybir.AxisListType
OP = mybir.AluOpType
ACT = mybir.ActivationFunctionType

# Full tiles whose elementwise multiply runs on GPSIMD instead of DVE.
GP_TILES = frozenset({1, 3, 5, 7, 9, 11})

TRACE = False                # set True (e.g. from test.py) to neuron-profile
LAST_RUN = None              # BassKernelResults of the most recent run

_CACHE = {}


def _build():
    nc = bacc.Bacc(trn_type="TRN2", enable_partition_id=False,
                   num_devices=NCORES)
    xs = nc.dram_tensor("xs", [BPC, N, D], F32, kind="ExternalInput").ap()
    tgt_d = nc.dram_tensor("tgtvec", [128, BPC], F32, kind="ExternalInput").ap()
    arep_d = nc.dram_tensor("arep", [128, K * D], F32, kind="ExternalInput").ap()
    mb_d = nc.dram_tensor("mbgrid", [128, COLS], F32, kind="ExternalInput").ap()
    id_d = nc.dram_tensor("ident", [128, 128], F32, kind="ExternalInput").ap()
    on_d = nc.dram_tensor("onesr", [1, 128], F32, kind="ExternalInput").ap()
    attn = nc.dram_tensor("attn", [BPC, N], F32, kind="ExternalOutput").ap()

    with tile.TileContext(nc) as tc, ExitStack() as ctx:
        singles = ctx.enter_context(tc.tile_pool(name="singles", bufs=1))
        xpool = ctx.enter_context(tc.tile_pool(name="xpool", bufs=4))
        ppool = ctx.enter_context(tc.tile_pool(name="ppool", bufs=3))
        spool = ctx.enter_context(tc.tile_pool(name="spool", bufs=2))
        epool = ctx.enter_context(tc.tile_pool(name="epool", bufs=2))
        stat = ctx.enter_context(tc.tile_pool(name="stat", bufs=8))
        pspool = ctx.enter_context(tc.tile_pool(name="ps", bufs=4, space="PSUM"))

        arep_sb = singles.tile([128, K * D], F32)
        nc.sync.dma_start(out=arep_sb, in_=arep_d)
        arep3 = arep_sb[:].rearrange("p (k d) -> p k d", d=D)
        mb_sb = singles.tile([128, COLS], F32)
        nc.sync.dma_start(out=mb_sb, in_=mb_d)
        tgt_sb = singles.tile([128, BPC], F32)
        nc.sync.dma_start(out=tgt_sb, in_=tgt_d)
        ident = singles.tile([128, 128], F32)
        nc.sync.dma_start(out=ident, in_=id_d)
        onesr = singles.tile([1, 128], F32)
        nc.sync.dma_start(out=onesr, in_=on_d)

        def cross_partition(vec, op):
            """[128,1] per-partition stats -> [1,1] global (PE transpose)."""
            tp = pspool.tile([1, 128], F32, tag="ps")
            nc.tensor.transpose(tp, vec, ident)
            ct = stat.tile([1, 128], F32, tag="ct")
            nc.vector.tensor_copy(ct, tp)
            g1 = stat.tile([1, 1], F32, tag="g1")
            nc.vector.tensor_reduce(g1, ct, axis=AX.X, op=op)
            return g1

        def bcast_partitions(s1, tag):
            """[1,1] scalar -> [128,1] replicated (ones-matmul)."""
            bp = pspool.tile([128, 1], F32, tag="ps")
            nc.tensor.matmul(bp, onesr, s1, start=True, stop=True)
            out = stat.tile([128, 1], F32, tag=tag)
            nc.vector.tensor_copy(out, bp)
            return out

        for b in range(BPC):
            sb = spool.tile([128, COLS], F32)
            # tail-tile slots with no node behind them: keep them finite so
            # the masked add (-9e15) sends them to zero probability.  (The
            # tail reduce overwrites partitions < PT afterwards; engines can
            # only start at partition 0/32/64/96, so clear the full block.)
            nc.vector.memset(sb[:, TFULL * K:], 0.0)
            for t in range(TFULL):
                xt = xpool.tile([128, K, D], F32)
                nc.sync.dma_start(
                    out=xt,
                    in_=xs[b, t * 128 * K:(t + 1) * 128 * K, :]
                        .rearrange("(p k) d -> p k d", p=128),
                )
                pr = ppool.tile([128, K, D], F32)
                eng = nc.gpsimd if t in GP_TILES else nc.vector
                eng.tensor_mul(pr, xt, arep3)
                nc.vector.reduce_sum(sb[:, t * K:(t + 1) * K], pr, axis=AX.X)
            # tail tile: 848 nodes = [53 partitions, 16 nodes, 64 feats]
            xt_t = xpool.tile([128, KT, D], F32)
            nc.sync.dma_start(
                out=xt_t[:PT],
                in_=xs[b, NFULL:N, :].rearrange("(p k) d -> p k d", p=PT),
            )
            pr_t = ppool.tile([128, KT, D], F32)
            nc.vector.tensor_mul(pr_t[:PT], xt_t[:PT], arep3[:PT, :KT, :])
            nc.vector.reduce_sum(sb[:PT, TFULL * K:], pr_t[:PT], axis=AX.X)

            # z = leaky_relu(scores + tgt, 0.01) + mask_bias
            z = epool.tile([128, COLS], F32)
            nc.vector.tensor_scalar_add(z, sb, tgt_sb[:, b:b + 1])
            nc.vector.scalar_tensor_tensor(z, z, 0.01, z,
                                           op0=OP.mult, op1=OP.max)
            nc.vector.tensor_add(z, z, mb_sb)

            pmax = stat.tile([128, 1], F32)
            nc.vector.reduce_max(pmax, z, axis=AX.X)
            gmax1 = cross_partition(pmax, OP.max)
            nmax1 = stat.tile([1, 1], F32)
            nc.vector.tensor_scalar_mul(nmax1, gmax1, -1.0)
            nmax = bcast_partitions(nmax1, "nmax")

            pb = epool.tile([128, COLS], F32)
            srow = stat.tile([128, 1], F32)
            nc.scalar.activation(pb, z, ACT.Exp, bias=nmax, scale=1.0,
                                 accum_out=srow)
            gsum1 = cross_partition(srow, OP.add)
            rec1 = stat.tile([1, 1], F32)
            nc.vector.reciprocal(rec1, gsum1)
            rec = bcast_partitions(rec1, "rec")
            nc.vector.tensor_scalar_mul(pb, pb, rec)

            nc.sync.dma_start(
                out=attn[b, 0:NFULL].rearrange("(t p k) -> p t k", p=128, k=K),
                in_=pb[:, 0:TFULL * K].rearrange("p (t k) -> p t k", t=TFULL),
            )
            nc.sync.dma_start(
                out=attn[b, NFULL:N].rearrange("(p k) -> p k", k=KT),
                in_=pb[:PT, TFULL * K:],
            )
    nc.compile()
    return nc


def _host_prep(x, a, node_index, adj_mask):
    x = np.asarray(x, dtype=np.float32)
    a = np.asarray(a, dtype=np.float32).reshape(2 * D)
    adj = np.asarray(adj_mask).astype(np.int64)
    idx = int(node_index)
    a_tgt, a_src = a[:D], a[D:]

    tgt = (x[:, idx, :] @ a_tgt).astype(np.float32)          # [B]
    arep = np.tile(a_src, (128, K)).astype(np.float32)       # [128, K*D]

    mb = np.full((128, COLS), NEG, np.float32)
    m_full = adj[:NFULL].reshape(TFULL, 128, K)
    mb[:, :TFULL * K] = np.where(
        m_full.transpose(1, 0, 2).reshape(128, TFULL * K) > 0, 0.0, NEG)
    m_tail = adj[NFULL:].reshape(PT, KT)
    mb[:PT, TFULL * K:] = np.where(m_tail > 0, 0.0, NEG)
    ident = np.eye(128, dtype=np.float32)
    onesr = np.ones((1, 128), dtype=np.float32)
    return x, tgt, arep, mb, ident, onesr


def _in_maps(x, tgt, arep, mb, ident, onesr):
    maps = []
    for c in range(NCORES):
        tv = np.tile(tgt[c * BPC:(c + 1) * BPC][None, :],
                     (128, 1)).astype(np.float32)
        maps.append({
            "xs": np.ascontiguousarray(x[c * BPC:(c + 1) * BPC]),
            "tgtvec": tv,
            "arep": arep,
            "mbgrid": mb,
            "ident": ident,
            "onesr": onesr,
        })
    return maps


def kernel(x, a, node_index, adj_mask):
    global LAST_RUN
    prep = _host_prep(x, a, node_index, adj_mask)
    if "nc" not in _CACHE:
        _CACHE["nc"] = _build()
    nc = _CACHE["nc"]
    res = run_bass_kernel_spmd(nc, _in_maps(*prep),
                               list(range(NCORES)), trace=TRACE)
    LAST_RUN = res
    return np.concatenate([res.results[c]["attn"] for c in range(NCORES)],
                          axis=0)
